# revision 1
# baseline (speedup 1.0000x reference)
"""Dynamic depthwise 3x3 conv (per-pixel weights) on 8 Trainium2 NeuronCores.

Problem:
  x:            [4, 64, 256, 256]  f32
  conv_weights: [4, 576, 256, 256] f32  (= [4, 64ch * 9tap, 256, 256])
  out[n,c,h,w] = sum_k w[n, c*9+k, h, w] * xpad[n, c, h+ki, w+kj],  k=(ki,kj) row-major

Sharding: pure data parallel over (batch n, H-half) -> 8 shards of
  x [64, 130, 258] (host zero-padded, +1 halo row each side),
  w [64, 9, 128, 256], out [64, 128, 256].

On-core layout: partition p = hb*64 + c (hb in {0,1} picks a 64-row block,
c the channel). Free dim holds (h, w) with x rows padded to 258 so all nine
taps are pure free-dim AP offsets (dh*258 + dw). Loop over h-tiles of Rh
rows; per tap k: DVE multiply w_k with the shifted x view and accumulate.
"""

import sys

sys.path.insert(0, "/opt/trn_rl_repo")

import numpy as np

import concourse.bass as bass
import concourse.bacc as bacc
import concourse.tile as tile
from concourse import mybir
from concourse.bass_utils import run_bass_kernel_spmd

N, C, H, W = 4, 64, 256, 256
KW = 3
NCORES = 8
HH = H // 2          # rows per core
RB = HH // 2         # rows per partition block (64)
Rh = 8               # rows per h-tile
T = RB // Rh         # h-tiles per core
Wp = W + 2           # padded row width
F32 = mybir.dt.float32

_CACHE = {}


def _build():
    nc = bacc.Bacc("TRN2", target_bir_lowering=False, debug=False, num_devices=NCORES)
    x_in = nc.dram_tensor("x", [C, HH + 2, Wp], F32, kind="ExternalInput")
    w_in = nc.dram_tensor("w", [C, KW * KW, HH, W], F32, kind="ExternalInput")
    y_out = nc.dram_tensor("y", [C, HH, W], F32, kind="ExternalOutput")

    with tile.TileContext(nc) as tc:
        with (
            tc.tile_pool(name="xp", bufs=2) as xpool,
            tc.tile_pool(name="wp", bufs=2) as wpool,
            tc.tile_pool(name="op", bufs=2) as opool,
            tc.tile_pool(name="pp", bufs=1) as ppool,
        ):
            for t in range(T):
                xt = xpool.tile([128, (Rh + 2) * Wp], F32)
                wt = wpool.tile([128, KW * KW * Rh * W], F32)
                for hb in range(2):
                    r0 = hb * RB + t * Rh
                    nc.sync.dma_start(
                        out=xt[hb * 64:(hb + 1) * 64, :].rearrange(
                            "p (h w) -> p h w", h=Rh + 2, w=Wp
                        ),
                        in_=x_in[:, r0:r0 + Rh + 2, :],
                    )
                    nc.sync.dma_start(
                        out=wt[hb * 64:(hb + 1) * 64, :].rearrange(
                            "p (k h w) -> p k h w", k=KW * KW, h=Rh, w=W
                        ),
                        in_=w_in[:, :, r0:r0 + Rh, :],
                    )

                ot = opool.tile([128, Rh * W], F32)
                pt = ppool.tile([128, Rh * W], F32)
                xv = xt[:].rearrange("p (h w) -> p h w", h=Rh + 2, w=Wp)
                wv = wt[:].rearrange("p (k h w) -> p k h w", k=KW * KW, h=Rh, w=W)
                ov = ot[:].rearrange("p (h w) -> p h w", h=Rh, w=W)
                pv = pt[:].rearrange("p (h w) -> p h w", h=Rh, w=W)
                for k in range(KW * KW):
                    dh, dw = divmod(k, KW)
                    xs = xv[:, dh:dh + Rh, dw:dw + W]
                    if k == 0:
                        nc.vector.tensor_mul(ov, wv[:, k], xs)
                    else:
                        nc.vector.tensor_mul(pv, wv[:, k], xs)
                        nc.vector.tensor_add(ov, ov, pv)

                for hb in range(2):
                    r0 = hb * RB + t * Rh
                    nc.sync.dma_start(
                        out=y_out[:, r0:r0 + Rh, :],
                        in_=ot[hb * 64:(hb + 1) * 64, :].rearrange(
                            "p (h w) -> p h w", h=Rh, w=W
                        ),
                    )
    nc.compile()
    return nc


def _get_nc():
    if "nc" not in _CACHE:
        _CACHE["nc"] = _build()
    return _CACHE["nc"]


def kernel(x: np.ndarray, conv_weights: np.ndarray) -> np.ndarray:
    nc = _get_nc()
    x = np.asarray(x, dtype=np.float32)
    w5 = np.asarray(conv_weights, dtype=np.float32).reshape(N, C, KW * KW, H, W)
    xp = np.pad(x, ((0, 0), (0, 0), (1, 1), (1, 1)))

    in_maps = []
    for i in range(NCORES):
        n, hf = divmod(i, 2)
        xc = np.ascontiguousarray(xp[n, :, hf * HH:hf * HH + HH + 2, :])
        wc = np.ascontiguousarray(w5[n, :, :, hf * HH:(hf + 1) * HH, :])
        in_maps.append({"x": xc, "w": wc})

    res = run_bass_kernel_spmd(nc, in_maps, list(range(NCORES)))
    out = np.empty((N, C, H, W), dtype=np.float32)
    for i in range(NCORES):
        n, hf = divmod(i, 2)
        out[n, :, hf * HH:(hf + 1) * HH, :] = res.results[i]["y"]
    return out


# revision 3
# speedup vs baseline: 1.4614x; 1.4614x over previous
"""Dynamic depthwise 3x3 conv (per-pixel weights) on 8 Trainium2 NeuronCores.

Problem:
  x:            [4, 64, 256, 256]  f32
  conv_weights: [4, 576, 256, 256] f32  (= [4, 64ch * 9tap, 256, 256])
  out[n,c,h,w] = sum_k w[n, c*9+k, h, w] * xpad[n, c, h+ki, w+kj],  k=(ki,kj) row-major

Sharding: pure data parallel over (batch n, H-half) -> 8 shards.

On-core layout: partition p = hb*64 + c (hb in {0,1} picks a 64-row block,
c the channel). Free dim holds (h, w) with x rows padded to 258 so all nine
taps are pure free-dim AP offsets (dh*258 + dw). Loop over h-tiles of Rh
rows; per tap k: DVE multiply w_k with the shifted x view and accumulate.

All inputs/outputs are repacked on the host into per-tile-contiguous
[T, 128, bytes] blocks so every DMA is one sequential HBM stream with one
large contiguous descriptor per partition (scattered-read DMA measured at
~13 GB/s/engine vs ~27 for sequential).
"""

import sys

sys.path.insert(0, "/opt/trn_rl_repo")

import numpy as np

import concourse.bass as bass
import concourse.bacc as bacc
import concourse.tile as tile
from concourse import mybir
from concourse.bass_utils import run_bass_kernel_spmd

N, C, H, W = 4, 64, 256, 256
KW = 3
NCORES = 8
HH = H // 2          # rows per core
RB = HH // 2         # rows per partition block (64)
Rh = 8               # rows per h-tile
T = RB // Rh         # h-tiles per core
Wp = W + 2           # padded row width
XF = (Rh + 2) * Wp   # x tile free elems
WF = KW * KW * Rh * W
OF = Rh * W
F32 = mybir.dt.float32

_CACHE = {}


def _build():
    nc = bacc.Bacc("TRN2", target_bir_lowering=False, debug=False, num_devices=NCORES)
    x_in = nc.dram_tensor("x", [T, 128, XF], F32, kind="ExternalInput")
    w_in = nc.dram_tensor("w", [T, 128, WF], F32, kind="ExternalInput")
    y_out = nc.dram_tensor("y", [T, 128, OF], F32, kind="ExternalOutput")

    with tile.TileContext(nc) as tc:
        with (
            tc.tile_pool(name="xp", bufs=2) as xpool,
            tc.tile_pool(name="wp", bufs=2) as wpool,
            tc.tile_pool(name="op", bufs=2) as opool,
            tc.tile_pool(name="pp", bufs=1) as ppool,
        ):
            for t in range(T):
                xt = xpool.tile([128, XF], F32)
                wt = wpool.tile([128, WF], F32)
                nc.sync.dma_start(out=xt[:], in_=x_in[t])
                nc.sync.dma_start(out=wt[:], in_=w_in[t])

                ot = opool.tile([128, OF], F32)
                pt = ppool.tile([128, OF], F32)
                xv = xt[:].rearrange("p (h w) -> p h w", h=Rh + 2, w=Wp)
                wv = wt[:].rearrange("p (k h w) -> p k h w", k=KW * KW, h=Rh, w=W)
                ov = ot[:].rearrange("p (h w) -> p h w", h=Rh, w=W)
                pv = pt[:].rearrange("p (h w) -> p h w", h=Rh, w=W)
                for k in range(KW * KW):
                    dh, dw = divmod(k, KW)
                    xs = xv[:, dh:dh + Rh, dw:dw + W]
                    if k == 0:
                        nc.vector.tensor_mul(ov, wv[:, k], xs)
                    else:
                        nc.vector.tensor_mul(pv, wv[:, k], xs)
                        nc.vector.tensor_add(ov, ov, pv)

                nc.sync.dma_start(out=y_out[t], in_=ot[:])
    nc.compile()
    return nc


def _get_nc():
    if "nc" not in _CACHE:
        _CACHE["nc"] = _build()
    return _CACHE["nc"]


def _pack_core(xp_n: np.ndarray, w5_n: np.ndarray, hf: int):
    """Repack one core's shard into per-tile-contiguous DMA blocks.

    xp_n: [C, H+2, Wp] host-padded x for batch n; w5_n: [C, 9, H, W].
    Returns x_blocks [T, 128, XF], w_blocks [T, 128, WF].
    """
    xc = xp_n[:, hf * HH:hf * HH + HH + 2, :]          # [C, HH+2, Wp]
    wc = w5_n[:, :, hf * HH:(hf + 1) * HH, :]          # [C, 9, HH, W]

    xb = np.empty((T, 2, C, Rh + 2, Wp), dtype=np.float32)
    for t in range(T):
        for hb in range(2):
            r0 = hb * RB + t * Rh
            xb[t, hb] = xc[:, r0:r0 + Rh + 2, :]
    # w: h = (hb, t, h_sub) -> [T, hb, C, 9, Rh, W]
    wb = (
        wc.reshape(C, KW * KW, 2, T, Rh, W)
        .transpose(3, 2, 0, 1, 4, 5)
        .reshape(T, 128, WF)
    )
    return xb.reshape(T, 128, XF), np.ascontiguousarray(wb)


def _make_in_maps(x: np.ndarray, conv_weights: np.ndarray):
    x = np.asarray(x, dtype=np.float32)
    w5 = np.asarray(conv_weights, dtype=np.float32).reshape(N, C, KW * KW, H, W)
    xp = np.pad(x, ((0, 0), (0, 0), (1, 1), (1, 1)))

    in_maps = []
    for i in range(NCORES):
        n, hf = divmod(i, 2)
        xb, wb = _pack_core(xp[n], w5[n], hf)
        in_maps.append({"x": xb, "w": wb})
    return in_maps


def kernel(x: np.ndarray, conv_weights: np.ndarray) -> np.ndarray:
    nc = _get_nc()
    in_maps = _make_in_maps(x, conv_weights)
    res = run_bass_kernel_spmd(nc, in_maps, list(range(NCORES)))
    out = np.empty((N, C, H, W), dtype=np.float32)
    for i in range(NCORES):
        n, hf = divmod(i, 2)
        yb = res.results[i]["y"].reshape(T, 2, C, Rh, W)
        # invert: out rows h = hf*HH + hb*RB + t*Rh + h_sub
        oc = yb.transpose(2, 1, 0, 3, 4).reshape(C, HH, W)
        out[n, :, hf * HH:(hf + 1) * HH, :] = oc
    return out


# revision 4
# speedup vs baseline: 1.8883x; 1.2921x over previous
"""Dynamic depthwise 3x3 conv (per-pixel weights) on 8 Trainium2 NeuronCores.

Problem:
  x:            [4, 64, 256, 256]  f32
  conv_weights: [4, 576, 256, 256] f32  (= [4, 64ch * 9tap, 256, 256])
  out[n,c,h,w] = sum_k w[n, c*9+k, h, w] * xpad[n, c, h+ki, w+kj],  k=(ki,kj) row-major

Sharding: pure data parallel over (batch n, H-half) -> 8 shards.

On-core layout: partition p = hb*64 + c (hb in {0,1} picks a 64-row block,
c the channel). Free dim holds (h, w) with x rows padded to 258 so all nine
taps are pure free-dim AP offsets (dh*258 + dw). Loop over h-tiles of Rh
rows; per tap k: DVE multiply w_k with the shifted x view and accumulate.

All inputs/outputs are repacked on the host into per-tile-contiguous
[T, 128, bytes] blocks so every DMA is one sequential HBM stream with one
large contiguous descriptor per partition (scattered-read DMA measured at
~13 GB/s/engine vs ~27 for sequential).
"""

import sys

sys.path.insert(0, "/opt/trn_rl_repo")

import numpy as np

import concourse.bass as bass
import concourse.bacc as bacc
import concourse.tile as tile
from concourse import mybir
from concourse.bass_utils import run_bass_kernel_spmd

N, C, H, W = 4, 64, 256, 256
KW = 3
NCORES = 8
HH = H // 2          # rows per core
RB = HH // 2         # rows per partition block (64)
Rh = 4               # rows per h-tile
T = RB // Rh         # h-tiles per core
Wp = W + 2           # padded row width
XF = (Rh + 2) * Wp   # x tile free elems
WF = KW * KW * Rh * W
OF = Rh * W
F32 = mybir.dt.float32

_CACHE = {}


def _build():
    from segmac import get_segmac_op, window_ap

    op = get_segmac_op()
    nc = bacc.Bacc("TRN2", target_bir_lowering=False, debug=False, num_devices=NCORES)
    x_in = nc.dram_tensor("x", [T, 128, XF], F32, kind="ExternalInput")
    w_in = nc.dram_tensor("w", [T, 128, WF], F32, kind="ExternalInput")
    y_out = nc.dram_tensor("y", [T, 128, OF], F32, kind="ExternalOutput")

    with tile.TileContext(nc) as tc:
        with (
            tc.tile_pool(name="xp", bufs=3) as xpool,
            tc.tile_pool(name="wp", bufs=3) as wpool,
            tc.tile_pool(name="op", bufs=2) as opool,
            tc.tile_pool(name="pa", bufs=1) as papool,
            tc.tile_pool(name="pb", bufs=1) as pbpool,
        ):
            for t in range(T):
                xt = xpool.tile([128, XF], F32)
                wt = wpool.tile([128, WF], F32)
                nc.sync.dma_start(out=xt[:], in_=x_in[t])
                nc.sync.dma_start(out=wt[:], in_=w_in[t])

                ot = opool.tile([128, OF], F32)
                pa = papool.tile([128, OF], F32)
                pb = pbpool.tile([128, OF], F32)
                # per (dh, row): 3-tap segmented MAC
                #   target[p, r*W + wd] = sum_dw w[(dh*3+dw), r, wd] * x[r+dh, wd+dw]
                for dh, tgt in ((0, ot), (1, pa), (2, pb)):
                    for r in range(Rh):
                        w_sl = wt[:, dh * 3 * Rh * W + r * W:
                                  dh * 3 * Rh * W + r * W + 2 * Rh * W + W]
                        x_sl = xt[:, (r + dh) * Wp:(r + dh) * Wp + Wp]
                        o_sl = tgt[:, r * W:(r + 1) * W]
                        nc.vector._custom_dve(
                            op,
                            out=window_ap(o_sl, [[1, W], [0, KW]]),
                            in0=window_ap(w_sl, [[1, W], [Rh * W, KW]]),
                            in1=window_ap(x_sl, [[1, W], [1, KW]]),
                        )
                nc.vector.tensor_add(ot[:], ot[:], pa[:])
                nc.vector.tensor_add(ot[:], ot[:], pb[:])

                nc.sync.dma_start(out=y_out[t], in_=ot[:])
    nc.compile()
    return nc


def _get_nc():
    if "nc" not in _CACHE:
        _CACHE["nc"] = _build()
    return _CACHE["nc"]


def _pack_core(xp_n: np.ndarray, w5_n: np.ndarray, hf: int):
    """Repack one core's shard into per-tile-contiguous DMA blocks.

    xp_n: [C, H+2, Wp] host-padded x for batch n; w5_n: [C, 9, H, W].
    Returns x_blocks [T, 128, XF], w_blocks [T, 128, WF].
    """
    xc = xp_n[:, hf * HH:hf * HH + HH + 2, :]          # [C, HH+2, Wp]
    wc = w5_n[:, :, hf * HH:(hf + 1) * HH, :]          # [C, 9, HH, W]

    xb = np.empty((T, 2, C, Rh + 2, Wp), dtype=np.float32)
    for t in range(T):
        for hb in range(2):
            r0 = hb * RB + t * Rh
            xb[t, hb] = xc[:, r0:r0 + Rh + 2, :]
    # w: h = (hb, t, h_sub) -> [T, hb, C, 9, Rh, W]
    wb = (
        wc.reshape(C, KW * KW, 2, T, Rh, W)
        .transpose(3, 2, 0, 1, 4, 5)
        .reshape(T, 128, WF)
    )
    return xb.reshape(T, 128, XF), np.ascontiguousarray(wb)


def _make_in_maps(x: np.ndarray, conv_weights: np.ndarray):
    x = np.asarray(x, dtype=np.float32)
    w5 = np.asarray(conv_weights, dtype=np.float32).reshape(N, C, KW * KW, H, W)
    xp = np.pad(x, ((0, 0), (0, 0), (1, 1), (1, 1)))

    in_maps = []
    for i in range(NCORES):
        n, hf = divmod(i, 2)
        xb, wb = _pack_core(xp[n], w5[n], hf)
        in_maps.append({"x": xb, "w": wb})
    return in_maps


def kernel(x: np.ndarray, conv_weights: np.ndarray) -> np.ndarray:
    nc = _get_nc()
    in_maps = _make_in_maps(x, conv_weights)
    res = run_bass_kernel_spmd(nc, in_maps, list(range(NCORES)))
    out = np.empty((N, C, H, W), dtype=np.float32)
    for i in range(NCORES):
        n, hf = divmod(i, 2)
        yb = res.results[i]["y"].reshape(T, 2, C, Rh, W)
        # invert: out rows h = hf*HH + hb*RB + t*Rh + h_sub
        oc = yb.transpose(2, 1, 0, 3, 4).reshape(C, HH, W)
        out[n, :, hf * HH:(hf + 1) * HH, :] = oc
    return out
